# revision 20
# baseline (speedup 1.0000x reference)
"""CorrLookup Trainium2 kernel (8 NeuronCores, SPMD data-parallel over pixels).

Reference op: for each pixel n (N = B*H*W = 16384) and each pyramid level l,
bilinear-sample an 81-point (9x9, radius 4) window centered at
(x_n + flow_x)/2^l from that pixel's own (H_l, W_l) correlation map, with
zero padding outside the map. Output (B, 4*81, H, W) f32.

Strategy per core (2048 pixels, pixel-per-partition, 16 pixels/partition):
  - Host precomputes, per pixel per level: span-start gather index, and the
    separable masked bilinear weights (y-taps w0/w1[9], x-taps v0/v1[9], edge
    masks folded in), all in bf16.
  - Corr maps ship as bf16, x-major ([x][y], contiguous span = 9*colstride+10
    covers the 10x10 footprint). Levels 0/1 additionally use overlapping
    row-bands (Hb=28, stride 19) so the span shrinks to 262 elements.
  - Levels 0-2 gather via SWDGE indirect DMA, 16 waves per level (HW limit:
    one dynamic offset per partition per instruction), alternating between
    two SWDGE queues. Level 3 instead loads its full 8x16 maps into SBUF
    with one regular DMA and extracts 10x10 footprints with a single
    local_scatter (per-partition int16 target indices, OOB slots -> 0).
  - ACT expands x-tap weights along the inner axis (stride-0 broadcast Copy)
    so every DVE tensor_tensor runs in bf16 2x_1p mode; the separable mix is
    6 ops per level-half (two g-halves so the tail overlaps final gathers).
  - Outputs written bf16 per half, host converts/reassembles to f32.
"""

import os
import sys
import types
import numpy as np
import ml_dtypes

bf16 = ml_dtypes.bfloat16

B, H, W = 2, 64, 128
N = B * H * W
N_CORES = 8
NPX = N // N_CORES  # 2048
GPP = NPX // 128  # 16 pixels per partition
LV = [(64, 128), (32, 64), (16, 32), (8, 16)]  # (Hc, Wc) per level
SBAND, HB = 19, 28
FRONT, BACK = 512, 1024
# per-level: (kind, colstride, nbands, block_els, span_els, gather_elem)
LAYOUT = []
for _l, (_Hc, _Wc) in enumerate(LV):
    if _Hc > HB or _Hc == 32:
        _nb = (_Hc - 1) // SBAND + 1
        LAYOUT.append(("band", HB, _nb, _nb * _Wc * HB, 9 * HB + 10, 9 * HB + 10))
    else:
        _sp = 9 * _Hc + 10
        _ge = 256 if _l == 2 else _sp  # pad L2 elem to 512B
        LAYOUT.append(("flat", _Hc, 1, _Hc * _Wc, _sp, _ge))
TOT = [FRONT + NPX * LAYOUT[l][3] + BACK for l in range(4)]
# L3 uses full-map SBUF + local_scatter into a 10x10 footprint (ge=100, cs=10)
LAYOUT[3] = ("scat", 10, 1, 128, 100, 100)
LORDER = [2, 3, 1, 0]  # Pool order: L2 waves, L3 scatter, L1, L0
LAST_EXEC_NS = None

_prog = None


def _install_trace_shim():
    try:
        import antenv

        if "antenv.axon_hooks" not in sys.modules:
            mod = types.ModuleType("antenv.axon_hooks")
            _h = [None]
            mod.set_axon_ntff_profile_hook = lambda hk: _h.__setitem__(0, hk)
            mod.get_axon_ntff_profile_hook = lambda: _h[0]
            sys.modules["antenv.axon_hooks"] = mod
            antenv.axon_hooks = mod
        from antenv.axon_hooks import set_axon_ntff_profile_hook

        from trn_agent_boot.trn_boot import _ntff_profile_via_ctypes

        set_axon_ntff_profile_hook(
            _ntff_profile_via_ctypes("/opt/axon/libaxon_pjrt.so")
        )
        import concourse.bass_utils as bu

        bu.upload_artifacts = lambda tmpdir: f"file://{tmpdir}"
        return True
    except Exception:
        return False


def _build():
    import concourse.bacc as bacc
    import concourse.bass as bass
    import concourse.tile as tile
    import concourse.mybir as mybir

    bft = mybir.dt.bfloat16
    i32 = mybir.dt.int32
    Alu = mybir.AluOpType
    Act = mybir.ActivationFunctionType

    nc = bacc.Bacc("TRN2", target_bir_lowering=False, debug=False, num_devices=N_CORES, num_swdge_queues=4)

    srcs = [
        nc.dram_tensor(f"src{l}", [TOT[l], 1], bft, kind="ExternalInput").ap()
        for l in range(3)
    ]
    src3f = nc.dram_tensor("src3f", [128, GPP * 128], bft, kind="ExternalInput").ap()
    idx3s = nc.dram_tensor(
        "idx3s", [128, GPP * 128], mybir.dt.int16, kind="ExternalInput"
    ).ap()
    idxd = nc.dram_tensor("idx", [128, 4 * GPP], i32, kind="ExternalInput").ap()
    w01d = nc.dram_tensor("w01", [128, 4 * GPP * 18], bft, kind="ExternalInput").ap()
    v01d = nc.dram_tensor("v01", [128, 4 * GPP * 18], bft, kind="ExternalInput").ap()
    outs = [
        nc.dram_tensor(f"out{l}", [128, GPP * 81], bft, kind="ExternalOutput").ap()
        for l in range(4)
    ]

    def AP(tile_ap, off_extra, dims):
        base = tile_ap
        return bass.AP(base.tensor, base.offset + off_extra, [list(base.ap[0])] + dims)

    with tile.TileContext(nc) as tc:
        with (
            tc.tile_pool(name="const", bufs=1) as cp,
            tc.tile_pool(name="patch", bufs=1) as pp,
            tc.tile_pool(name="work", bufs=1) as wp,
        ):
            idx_t = cp.tile([128, 4 * GPP], i32)
            w01_t = cp.tile([128, 4 * GPP * 18], bft)
            v01_t = cp.tile([128, 4 * GPP * 18], bft)
            s3f_t = cp.tile([128, GPP * 128], bft)
            i3s_t = cp.tile([128, GPP * 128], mybir.dt.int16)
            nc.sync.dma_start(out=idx_t[:], in_=idxd)
            nc.sync.dma_start(out=s3f_t[:], in_=src3f)
            nc.sync.dma_start(out=i3s_t[:], in_=idx3s)
            nc.sync.dma_start(out=w01_t[:], in_=w01d)
            nc.sync.dma_start(out=v01_t[:], in_=v01d)

            # gathers: 16 waves per level (HW supports 1 offset/partition/DMA);
            # L3: full maps in SBUF + one local_scatter to footprints, emitted
            # after only 4 L2 waves (inputs have landed, few transfers in
            # flight to stall the ISA op)
            patch = {}
            for l in (2, 1, 0, 3):
                ge = LAYOUT[l][5]
                pt = pp.tile([128, GPP * ge], bft, tag=f"patch{l}")
                patch[l] = pt

            def emit_scatter():
                nc.gpsimd.local_scatter(
                    out_ap=patch[3][:],
                    data_ap=s3f_t[:],
                    idxs_ap=i3s_t[:],
                    channels=128,
                    num_elems=GPP * 100,
                    num_idxs=GPP * 128,
                )

            for l in (2, 1, 0):
                ge = LAYOUT[l][5]
                pt = patch[l]
                for w in range(GPP):
                    gi = nc.gpsimd.indirect_dma_start(
                        out=pt[:, w * ge : (w + 1) * ge],
                        out_offset=None,
                        in_=srcs[l],
                        in_offset=bass.IndirectOffsetOnAxis(
                            ap=idx_t[:, l * GPP + w : l * GPP + w + 1], axis=0
                        ),
                    )
                    if w % 4:
                        gi.ins.queue = f"qPoolDynamic{w % 4}"
                    if l == 2 and w == 3:
                        emit_scatter()

            # ACT: expand x-tap weights v01[g,d,i] -> v01e[g,d,i,j] (j bcast)
            v01e = {}
            for l in LORDER:
                ve = wp.tile([128, GPP * 162], bft, tag=f"v01e{l}")
                nc.scalar.activation(
                    out=AP(ve[:], 0, [[81, GPP * 2], [9, 9], [1, 9]]),
                    in_=AP(v01_t[:], l * GPP * 18, [[9, GPP * 2], [1, 9], [0, 9]]),
                    func=Act.Copy,
                )
                v01e[l] = ve

            # DVE: separable masked bilinear mix per level (all ops bf16 2x),
            # in two g-halves so the tail mix overlaps the final gathers
            for l in LORDER:
                cs, ge = LAYOUT[l][1], LAYOUT[l][5]
                pt = patch[l]
                nparts = {0: 4}.get(l, 1)
                HG = GPP // nparts
                for h in range(nparts):
                    g0 = h * HG
                    wof = l * GPP * 18 + g0 * 18
                    t1 = wp.tile([128, HG * 90], bft, tag=f"t1{l}{h}")
                    t2 = wp.tile([128, HG * 90], bft, tag=f"t2{l}{h}")
                    qb = wp.tile([128, HG * 90], bft, tag=f"qb{l}{h}")
                    # t1[g,a,b] = P[g,a,b]*w0[g,b]; t2[g,a,b] = P[g,a,b+1]*w1[g,b]
                    nc.vector.tensor_tensor(
                        out=AP(t1[:], 0, [[90, HG], [9, 10], [1, 9]]),
                        in0=AP(pt[:], g0 * ge, [[ge, HG], [cs, 10], [1, 9]]),
                        in1=AP(w01_t[:], wof, [[18, HG], [0, 10], [1, 9]]),
                        op=Alu.mult,
                    )
                    nc.vector.tensor_tensor(
                        out=AP(t2[:], 0, [[90, HG], [9, 10], [1, 9]]),
                        in0=AP(pt[:], g0 * ge + 1, [[ge, HG], [cs, 10], [1, 9]]),
                        in1=AP(w01_t[:], wof + 9, [[18, HG], [0, 10], [1, 9]]),
                        op=Alu.mult,
                    )
                    nc.vector.tensor_tensor(
                        out=qb[:], in0=t1[:], in1=t2[:], op=Alu.add
                    )
                    u1 = wp.tile([128, HG * 81], bft, tag=f"u1{l}{h}")
                    u2 = wp.tile([128, HG * 81], bft, tag=f"u2{l}{h}")
                    ot = wp.tile([128, HG * 81], bft, tag=f"ot{l}{h}")
                    vof = g0 * 162
                    # u1[g,i,j] = qb[g,i,j]*v0e; u2[g,i,j] = qb[g,i+1,j]*v1e
                    nc.vector.tensor_tensor(
                        out=AP(u1[:], 0, [[81, HG], [9, 9], [1, 9]]),
                        in0=AP(qb[:], 0, [[90, HG], [9, 9], [1, 9]]),
                        in1=AP(v01e[l][:], vof, [[162, HG], [9, 9], [1, 9]]),
                        op=Alu.mult,
                    )
                    nc.vector.tensor_tensor(
                        out=AP(u2[:], 0, [[81, HG], [9, 9], [1, 9]]),
                        in0=AP(qb[:], 9, [[90, HG], [9, 9], [1, 9]]),
                        in1=AP(v01e[l][:], vof + 81, [[162, HG], [9, 9], [1, 9]]),
                        op=Alu.mult,
                    )
                    nc.vector.tensor_tensor(
                        out=ot[:], in0=u1[:], in1=u2[:], op=Alu.add
                    )
                    nc.sync.dma_start(
                        out=bass.AP(
                            outs[l].tensor,
                            outs[l].offset + g0 * 81,
                            [list(outs[l].ap[0]), [1, HG * 81]],
                        ),
                        in_=ot[:],
                    )

    nc.compile()
    return nc


def _host_precompute(flow):
    """Per level: gather idx [N] i64, w01 [N,2,9] f32, v01 [N,2,9] f32."""
    fl = np.asarray(flow, dtype=np.float32).transpose(0, 2, 3, 1).reshape(N, 2)
    xg = np.tile(np.arange(W, dtype=np.float32), H * B)
    yg = np.tile(np.repeat(np.arange(H, dtype=np.float32), W), B)
    res = []
    for l, (Hc, Wc) in enumerate(LV):
        kind, cs, nb, block, span, ge = LAYOUT[l]
        s = np.float32(1.0 / (1 << l))
        Cx = ((xg + fl[:, 0]) * s).astype(np.float64)
        Cy = ((yg + fl[:, 1]) * s).astype(np.float64)
        x0 = np.floor(Cx)
        y0 = np.floor(Cy)
        wx = (Cx - x0).astype(np.float32)
        wy = (Cy - y0).astype(np.float32)
        x0 = x0.astype(np.int64)
        y0 = y0.astype(np.int64)
        a = np.arange(10)
        mx = (((x0[:, None] - 4 + a) >= 0) & ((x0[:, None] - 4 + a) <= Wc - 1)).astype(
            np.float32
        )
        my = (((y0[:, None] - 4 + a) >= 0) & ((y0[:, None] - 4 + a) <= Hc - 1)).astype(
            np.float32
        )
        w01 = np.stack(
            [(1 - wy)[:, None] * my[:, :9], wy[:, None] * my[:, 1:]], axis=1
        )
        v01 = np.stack(
            [(1 - wx)[:, None] * mx[:, :9], wx[:, None] * mx[:, 1:]], axis=1
        )
        n_loc = np.arange(N, dtype=np.int64) % NPX
        if kind == "band":
            Bb = np.clip((y0 - 4) // SBAND, 0, nb - 1)
            idx = (
                FRONT
                + n_loc * block
                + Bb * (Wc * HB)
                + (x0 - 4) * HB
                + (y0 - 4 - Bb * SBAND)
            )
        else:
            idx = FRONT + n_loc * block + (x0 - 4) * Hc + (y0 - 4)
        idx = np.clip(idx, 0, TOT[l] - ge)
        res.append((idx, w01, v01, x0, y0))
    return res


def _build_src(corr, l):
    """corr: (N, Hc, Wc) f32 for this level -> per-core list of bf16 buffers."""
    kind, cs, nb, block, span, ge = LAYOUT[l]
    Hc, Wc = LV[l]
    bufs = []
    for c in range(N_CORES):
        shard = corr[c * NPX : (c + 1) * NPX]  # (NPX, Hc, Wc)
        tr = np.ascontiguousarray(shard.transpose(0, 2, 1))  # [px][x][y]
        if kind == "band":
            banded = np.zeros((NPX, nb, Wc, HB), dtype=bf16)
            for b in range(nb):
                y0 = b * SBAND
                y1 = min(y0 + HB, Hc)
                banded[:, b, :, : y1 - y0] = tr[:, :, y0:y1].astype(bf16)
            flat = banded.reshape(NPX, -1)
        else:
            flat = tr.reshape(NPX, -1).astype(bf16)
        buf = np.zeros(TOT[l], dtype=bf16)
        buf[FRONT : FRONT + NPX * block] = flat.reshape(-1)
        bufs.append(buf.reshape(-1, 1))
    return bufs


def _marshal(corr0, corr1, corr2, corr3, flow):
    corrs = [corr0, corr1, corr2, corr3]
    pre = _host_precompute(flow)
    in_maps = [dict() for _ in range(N_CORES)]
    for l in range(3):
        Hc, Wc = LV[l]
        srcs = _build_src(np.asarray(corrs[l], dtype=np.float32).reshape(N, Hc, Wc), l)
        for c in range(N_CORES):
            in_maps[c][f"src{l}"] = srcs[c]
    # L3: full transposed maps per (partition, g-slot) + scatter target indices
    Hc, Wc = LV[3]
    m3 = np.asarray(corrs[3], dtype=np.float32).reshape(N, Hc, Wc)
    m3t = np.ascontiguousarray(m3.transpose(0, 2, 1)).reshape(N, Wc * Hc)  # [x][y]
    _, _, _, x0_3, y0_3 = pre[3]
    xe = np.arange(Wc * Hc) // Hc  # element x
    ye = np.arange(Wc * Hc) % Hc
    a3 = xe[None, :] - (x0_3[:, None] - 4)
    b3 = ye[None, :] - (y0_3[:, None] - 4)
    tgt = np.where(
        (a3 >= 0) & (a3 < 10) & (b3 >= 0) & (b3 < 10), a3 * 10 + b3, -1
    ).astype(np.int64)  # (N, 128) in-pixel target or -1
    for c in range(N_CORES):
        lo = c * NPX
        # pixel (g,p) -> [p, g*128:(g+1)*128]
        mm = m3t[lo : lo + NPX].reshape(GPP, 128, Wc * Hc).transpose(1, 0, 2)
        in_maps[c]["src3f"] = np.ascontiguousarray(mm.reshape(128, -1)).astype(bf16)
        tt = tgt[lo : lo + NPX].reshape(GPP, 128, Wc * Hc).transpose(1, 0, 2).copy()
        goff = (np.arange(GPP) * 100)[None, :, None]
        tt = np.where(tt >= 0, tt + goff, -1)
        in_maps[c]["idx3s"] = np.ascontiguousarray(
            tt.reshape(128, -1).astype(np.int16)
        )
    # idx / weights: pixel (g, p) of core c = global c*NPX + g*128 + p
    idx_all = np.empty((N_CORES, 128, 4 * GPP), dtype=np.int32)
    w01_all = np.empty((N_CORES, 128, 4 * GPP * 18), dtype=bf16)
    v01_all = np.empty((N_CORES, 128, 4 * GPP * 18), dtype=bf16)
    for l in range(4):
        idx, w01, v01 = pre[l][:3]
        # reshape N -> (cores, g, p) -> (cores, p, g)
        idx_c = idx.reshape(N_CORES, GPP, 128).transpose(0, 2, 1)
        idx_all[:, :, l * GPP : (l + 1) * GPP] = idx_c.astype(np.int32)
        w_c = w01.reshape(N_CORES, GPP, 128, 18).transpose(0, 2, 1, 3)
        w01_all[:, :, l * GPP * 18 : (l + 1) * GPP * 18] = w_c.reshape(
            N_CORES, 128, -1
        ).astype(bf16)
        v_c = v01.reshape(N_CORES, GPP, 128, 18).transpose(0, 2, 1, 3)
        v01_all[:, :, l * GPP * 18 : (l + 1) * GPP * 18] = v_c.reshape(
            N_CORES, 128, -1
        ).astype(bf16)
    for c in range(N_CORES):
        in_maps[c]["idx"] = idx_all[c]
        in_maps[c]["w01"] = w01_all[c]
        in_maps[c]["v01"] = v01_all[c]
    return in_maps


def kernel(corr0, corr1, corr2, corr3, flow):
    global _prog, LAST_EXEC_NS
    trace = os.environ.get("CORR_TRACE") == "1"
    if trace:
        trace = _install_trace_shim()
    from concourse.bass_utils import run_bass_kernel_spmd

    if _prog is None:
        _prog = _build()
    in_maps = _marshal(corr0, corr1, corr2, corr3, flow)
    res = run_bass_kernel_spmd(
        _prog,
        in_maps,
        core_ids=list(range(N_CORES)),
        trace=trace,
        trace_cores=[0] if trace else None,
    )
    LAST_EXEC_NS = res.exec_time_ns
    if trace and res.instructions_and_trace:
        kernel.last_insts = res.instructions_and_trace
    full = np.empty((N, 324), dtype=np.float32)
    for c in range(N_CORES):
        lo = c * NPX
        for l in range(4):
            o = np.asarray(res.results[c][f"out{l}"]).astype(np.float32)
            o = o.reshape(128, GPP, 81)
            full[lo : lo + NPX, l * 81 : (l + 1) * 81] = (
                o.transpose(1, 0, 2).reshape(NPX, 81)
            )
    return np.ascontiguousarray(
        full.reshape(B, H, W, 324).transpose(0, 3, 1, 2)
    )


# revision 21
# speedup vs baseline: 1.0216x; 1.0216x over previous
"""CorrLookup Trainium2 kernel (8 NeuronCores, SPMD data-parallel over pixels).

Reference op: for each pixel n (N = B*H*W = 16384) and each pyramid level l,
bilinear-sample an 81-point (9x9, radius 4) window centered at
(x_n + flow_x)/2^l from that pixel's own (H_l, W_l) correlation map, with
zero padding outside the map. Output (B, 4*81, H, W) f32.

Strategy per core (2048 pixels, pixel-per-partition, 16 pixels/partition):
  - Host precomputes, per pixel per level: span-start gather index, and the
    separable masked bilinear weights (y-taps w0/w1[9], x-taps v0/v1[9], edge
    masks folded in), all in bf16.
  - Corr maps ship as bf16, x-major ([x][y], contiguous span = 9*colstride+10
    covers the 10x10 footprint). Levels 0/1 additionally use overlapping
    row-bands (Hb=28, stride 19) so the span shrinks to 262 elements.
  - Levels 0-2 gather via SWDGE indirect DMA, 16 waves per level (HW limit:
    one dynamic offset per partition per instruction), alternating between
    two SWDGE queues. Level 3 instead loads its full 8x16 maps into SBUF
    with one regular DMA and extracts 10x10 footprints with a single
    local_scatter (per-partition int16 target indices, OOB slots -> 0).
  - ACT expands x-tap weights along the inner axis (stride-0 broadcast Copy)
    so every DVE tensor_tensor runs in bf16 2x_1p mode; the separable mix is
    6 ops per level-half (two g-halves so the tail overlaps final gathers).
  - Outputs written bf16 per half, host converts/reassembles to f32.
"""

import os
import sys
import types
import numpy as np
import ml_dtypes

bf16 = ml_dtypes.bfloat16

B, H, W = 2, 64, 128
N = B * H * W
N_CORES = 8
NPX = N // N_CORES  # 2048
GPP = NPX // 128  # 16 pixels per partition
LV = [(64, 128), (32, 64), (16, 32), (8, 16)]  # (Hc, Wc) per level
SBAND, HB = 19, 28
FRONT, BACK = 512, 1024
# per-level: (kind, colstride, nbands, block_els, span_els, gather_elem)
LAYOUT = []
for _l, (_Hc, _Wc) in enumerate(LV):
    if _Hc > HB or _Hc == 32:
        _nb = (_Hc - 1) // SBAND + 1
        LAYOUT.append(("band", HB, _nb, _nb * _Wc * HB, 9 * HB + 10, 9 * HB + 10))
    else:
        _sp = 9 * _Hc + 10
        _ge = 256 if _l == 2 else _sp  # pad L2 elem to 512B
        LAYOUT.append(("flat", _Hc, 1, _Hc * _Wc, _sp, _ge))
TOT = [FRONT + NPX * LAYOUT[l][3] + BACK for l in range(4)]
# L3 uses full-map SBUF + local_scatter into a 10x10 footprint (ge=100, cs=10)
LAYOUT[3] = ("scat", 10, 1, 128, 100, 100)
LORDER = [2, 3, 1, 0]  # Pool order: L2 waves, L3 scatter, L1, L0
LAST_EXEC_NS = None

_prog = None


def _install_trace_shim():
    try:
        import antenv

        if "antenv.axon_hooks" not in sys.modules:
            mod = types.ModuleType("antenv.axon_hooks")
            _h = [None]
            mod.set_axon_ntff_profile_hook = lambda hk: _h.__setitem__(0, hk)
            mod.get_axon_ntff_profile_hook = lambda: _h[0]
            sys.modules["antenv.axon_hooks"] = mod
            antenv.axon_hooks = mod
        from antenv.axon_hooks import set_axon_ntff_profile_hook

        from trn_agent_boot.trn_boot import _ntff_profile_via_ctypes

        set_axon_ntff_profile_hook(
            _ntff_profile_via_ctypes("/opt/axon/libaxon_pjrt.so")
        )
        import concourse.bass_utils as bu

        bu.upload_artifacts = lambda tmpdir: f"file://{tmpdir}"
        return True
    except Exception:
        return False


def _build():
    import concourse.bacc as bacc
    import concourse.bass as bass
    import concourse.tile as tile
    import concourse.mybir as mybir

    bft = mybir.dt.bfloat16
    i32 = mybir.dt.int32
    Alu = mybir.AluOpType
    Act = mybir.ActivationFunctionType

    nc = bacc.Bacc("TRN2", target_bir_lowering=False, debug=False, num_devices=N_CORES, num_swdge_queues=4)

    srcs = [
        nc.dram_tensor(f"src{l}", [TOT[l], 1], bft, kind="ExternalInput").ap()
        for l in range(3)
    ]
    src3f = nc.dram_tensor("src3f", [128, GPP * 128], bft, kind="ExternalInput").ap()
    idx3s = nc.dram_tensor(
        "idx3s", [128, GPP * 128], mybir.dt.int16, kind="ExternalInput"
    ).ap()
    idxd = nc.dram_tensor("idx", [128, 4 * GPP], i32, kind="ExternalInput").ap()
    w01d = nc.dram_tensor("w01", [128, 4 * GPP * 18], bft, kind="ExternalInput").ap()
    v01d = nc.dram_tensor("v01", [128, 4 * GPP * 18], bft, kind="ExternalInput").ap()
    outs = [
        nc.dram_tensor(f"out{l}", [128, GPP * 81], bft, kind="ExternalOutput").ap()
        for l in range(4)
    ]

    def AP(tile_ap, off_extra, dims):
        base = tile_ap
        return bass.AP(base.tensor, base.offset + off_extra, [list(base.ap[0])] + dims)

    with tile.TileContext(nc) as tc:
        with (
            tc.tile_pool(name="const", bufs=1) as cp,
            tc.tile_pool(name="patch", bufs=1) as pp,
            tc.tile_pool(name="work", bufs=1) as wp,
        ):
            idx_t = cp.tile([128, 4 * GPP], i32)
            w01_t = cp.tile([128, 4 * GPP * 18], bft)
            v01_t = cp.tile([128, 4 * GPP * 18], bft)
            s3f_t = cp.tile([128, GPP * 128], bft)
            i3s_t = cp.tile([128, GPP * 128], mybir.dt.int16)
            nc.sync.dma_start(out=idx_t[:], in_=idxd)
            nc.sync.dma_start(out=s3f_t[:], in_=src3f)
            nc.sync.dma_start(out=i3s_t[:], in_=idx3s)
            nc.sync.dma_start(out=w01_t[:], in_=w01d)
            nc.sync.dma_start(out=v01_t[:], in_=v01d)

            # gathers: 16 waves per level (HW supports 1 offset/partition/DMA);
            # L3 instead: full maps in SBUF + local_scatter to footprints
            patch = {}
            for l in LORDER:
                ge = LAYOUT[l][5]
                pt = pp.tile([128, GPP * ge], bft, tag=f"patch{l}")
                if LAYOUT[l][0] == "scat":
                    nc.gpsimd.local_scatter(
                        out_ap=pt[:],
                        data_ap=s3f_t[:],
                        idxs_ap=i3s_t[:],
                        channels=128,
                        num_elems=GPP * 100,
                        num_idxs=GPP * 128,
                    )
                else:
                    for w in range(GPP):
                        gi = nc.gpsimd.indirect_dma_start(
                            out=pt[:, w * ge : (w + 1) * ge],
                            out_offset=None,
                            in_=srcs[l],
                            in_offset=bass.IndirectOffsetOnAxis(
                                ap=idx_t[:, l * GPP + w : l * GPP + w + 1], axis=0
                            ),
                        )
                        if w % 4:
                            gi.ins.queue = f"qPoolDynamic{w % 4}"
                patch[l] = pt

            # ACT: expand x-tap weights v01[g,d,i] -> v01e[g,d,i,j] (j bcast)
            v01e = {}
            for l in LORDER:
                ve = wp.tile([128, GPP * 162], bft, tag=f"v01e{l}")
                nc.scalar.activation(
                    out=AP(ve[:], 0, [[81, GPP * 2], [9, 9], [1, 9]]),
                    in_=AP(v01_t[:], l * GPP * 18, [[9, GPP * 2], [1, 9], [0, 9]]),
                    func=Act.Copy,
                )
                v01e[l] = ve

            # DVE: separable masked bilinear mix per level (all ops bf16 2x),
            # in two g-halves so the tail mix overlaps the final gathers
            for l in LORDER:
                cs, ge = LAYOUT[l][1], LAYOUT[l][5]
                pt = patch[l]
                nparts = 4 if l == 0 else 2
                HG = GPP // nparts
                for h in range(nparts):
                    g0 = h * HG
                    wof = l * GPP * 18 + g0 * 18
                    t1 = wp.tile([128, HG * 90], bft, tag=f"t1{l}{h}")
                    t2 = wp.tile([128, HG * 90], bft, tag=f"t2{l}{h}")
                    qb = wp.tile([128, HG * 90], bft, tag=f"qb{l}{h}")
                    # t1[g,a,b] = P[g,a,b]*w0[g,b]; t2[g,a,b] = P[g,a,b+1]*w1[g,b]
                    nc.vector.tensor_tensor(
                        out=AP(t1[:], 0, [[90, HG], [9, 10], [1, 9]]),
                        in0=AP(pt[:], g0 * ge, [[ge, HG], [cs, 10], [1, 9]]),
                        in1=AP(w01_t[:], wof, [[18, HG], [0, 10], [1, 9]]),
                        op=Alu.mult,
                    )
                    nc.vector.tensor_tensor(
                        out=AP(t2[:], 0, [[90, HG], [9, 10], [1, 9]]),
                        in0=AP(pt[:], g0 * ge + 1, [[ge, HG], [cs, 10], [1, 9]]),
                        in1=AP(w01_t[:], wof + 9, [[18, HG], [0, 10], [1, 9]]),
                        op=Alu.mult,
                    )
                    nc.vector.tensor_tensor(
                        out=qb[:], in0=t1[:], in1=t2[:], op=Alu.add
                    )
                    u1 = wp.tile([128, HG * 81], bft, tag=f"u1{l}{h}")
                    u2 = wp.tile([128, HG * 81], bft, tag=f"u2{l}{h}")
                    ot = wp.tile([128, HG * 81], bft, tag=f"ot{l}{h}")
                    vof = g0 * 162
                    # u1[g,i,j] = qb[g,i,j]*v0e; u2[g,i,j] = qb[g,i+1,j]*v1e
                    nc.vector.tensor_tensor(
                        out=AP(u1[:], 0, [[81, HG], [9, 9], [1, 9]]),
                        in0=AP(qb[:], 0, [[90, HG], [9, 9], [1, 9]]),
                        in1=AP(v01e[l][:], vof, [[162, HG], [9, 9], [1, 9]]),
                        op=Alu.mult,
                    )
                    nc.vector.tensor_tensor(
                        out=AP(u2[:], 0, [[81, HG], [9, 9], [1, 9]]),
                        in0=AP(qb[:], 9, [[90, HG], [9, 9], [1, 9]]),
                        in1=AP(v01e[l][:], vof + 81, [[162, HG], [9, 9], [1, 9]]),
                        op=Alu.mult,
                    )
                    nc.vector.tensor_tensor(
                        out=ot[:], in0=u1[:], in1=u2[:], op=Alu.add
                    )
                    nc.sync.dma_start(
                        out=bass.AP(
                            outs[l].tensor,
                            outs[l].offset + g0 * 81,
                            [list(outs[l].ap[0]), [1, HG * 81]],
                        ),
                        in_=ot[:],
                    )

    nc.compile()
    return nc


def _host_precompute(flow):
    """Per level: gather idx [N] i64, w01 [N,2,9] f32, v01 [N,2,9] f32."""
    fl = np.asarray(flow, dtype=np.float32).transpose(0, 2, 3, 1).reshape(N, 2)
    xg = np.tile(np.arange(W, dtype=np.float32), H * B)
    yg = np.tile(np.repeat(np.arange(H, dtype=np.float32), W), B)
    res = []
    for l, (Hc, Wc) in enumerate(LV):
        kind, cs, nb, block, span, ge = LAYOUT[l]
        s = np.float32(1.0 / (1 << l))
        Cx = ((xg + fl[:, 0]) * s).astype(np.float64)
        Cy = ((yg + fl[:, 1]) * s).astype(np.float64)
        x0 = np.floor(Cx)
        y0 = np.floor(Cy)
        wx = (Cx - x0).astype(np.float32)
        wy = (Cy - y0).astype(np.float32)
        x0 = x0.astype(np.int64)
        y0 = y0.astype(np.int64)
        a = np.arange(10)
        mx = (((x0[:, None] - 4 + a) >= 0) & ((x0[:, None] - 4 + a) <= Wc - 1)).astype(
            np.float32
        )
        my = (((y0[:, None] - 4 + a) >= 0) & ((y0[:, None] - 4 + a) <= Hc - 1)).astype(
            np.float32
        )
        w01 = np.stack(
            [(1 - wy)[:, None] * my[:, :9], wy[:, None] * my[:, 1:]], axis=1
        )
        v01 = np.stack(
            [(1 - wx)[:, None] * mx[:, :9], wx[:, None] * mx[:, 1:]], axis=1
        )
        n_loc = np.arange(N, dtype=np.int64) % NPX
        if kind == "band":
            Bb = np.clip((y0 - 4) // SBAND, 0, nb - 1)
            idx = (
                FRONT
                + n_loc * block
                + Bb * (Wc * HB)
                + (x0 - 4) * HB
                + (y0 - 4 - Bb * SBAND)
            )
        else:
            idx = FRONT + n_loc * block + (x0 - 4) * Hc + (y0 - 4)
        idx = np.clip(idx, 0, TOT[l] - ge)
        res.append((idx, w01, v01, x0, y0))
    return res


def _build_src(corr, l):
    """corr: (N, Hc, Wc) f32 for this level -> per-core list of bf16 buffers."""
    kind, cs, nb, block, span, ge = LAYOUT[l]
    Hc, Wc = LV[l]
    bufs = []
    for c in range(N_CORES):
        shard = corr[c * NPX : (c + 1) * NPX]  # (NPX, Hc, Wc)
        tr = np.ascontiguousarray(shard.transpose(0, 2, 1))  # [px][x][y]
        if kind == "band":
            banded = np.zeros((NPX, nb, Wc, HB), dtype=bf16)
            for b in range(nb):
                y0 = b * SBAND
                y1 = min(y0 + HB, Hc)
                banded[:, b, :, : y1 - y0] = tr[:, :, y0:y1].astype(bf16)
            flat = banded.reshape(NPX, -1)
        else:
            flat = tr.reshape(NPX, -1).astype(bf16)
        buf = np.zeros(TOT[l], dtype=bf16)
        buf[FRONT : FRONT + NPX * block] = flat.reshape(-1)
        bufs.append(buf.reshape(-1, 1))
    return bufs


def _marshal(corr0, corr1, corr2, corr3, flow):
    corrs = [corr0, corr1, corr2, corr3]
    pre = _host_precompute(flow)
    in_maps = [dict() for _ in range(N_CORES)]
    for l in range(3):
        Hc, Wc = LV[l]
        srcs = _build_src(np.asarray(corrs[l], dtype=np.float32).reshape(N, Hc, Wc), l)
        for c in range(N_CORES):
            in_maps[c][f"src{l}"] = srcs[c]
    # L3: full transposed maps per (partition, g-slot) + scatter target indices
    Hc, Wc = LV[3]
    m3 = np.asarray(corrs[3], dtype=np.float32).reshape(N, Hc, Wc)
    m3t = np.ascontiguousarray(m3.transpose(0, 2, 1)).reshape(N, Wc * Hc)  # [x][y]
    _, _, _, x0_3, y0_3 = pre[3]
    xe = np.arange(Wc * Hc) // Hc  # element x
    ye = np.arange(Wc * Hc) % Hc
    a3 = xe[None, :] - (x0_3[:, None] - 4)
    b3 = ye[None, :] - (y0_3[:, None] - 4)
    tgt = np.where(
        (a3 >= 0) & (a3 < 10) & (b3 >= 0) & (b3 < 10), a3 * 10 + b3, -1
    ).astype(np.int64)  # (N, 128) in-pixel target or -1
    for c in range(N_CORES):
        lo = c * NPX
        # pixel (g,p) -> [p, g*128:(g+1)*128]
        mm = m3t[lo : lo + NPX].reshape(GPP, 128, Wc * Hc).transpose(1, 0, 2)
        in_maps[c]["src3f"] = np.ascontiguousarray(mm.reshape(128, -1)).astype(bf16)
        tt = tgt[lo : lo + NPX].reshape(GPP, 128, Wc * Hc).transpose(1, 0, 2).copy()
        goff = (np.arange(GPP) * 100)[None, :, None]
        tt = np.where(tt >= 0, tt + goff, -1)
        in_maps[c]["idx3s"] = np.ascontiguousarray(
            tt.reshape(128, -1).astype(np.int16)
        )
    # idx / weights: pixel (g, p) of core c = global c*NPX + g*128 + p
    idx_all = np.empty((N_CORES, 128, 4 * GPP), dtype=np.int32)
    w01_all = np.empty((N_CORES, 128, 4 * GPP * 18), dtype=bf16)
    v01_all = np.empty((N_CORES, 128, 4 * GPP * 18), dtype=bf16)
    for l in range(4):
        idx, w01, v01 = pre[l][:3]
        # reshape N -> (cores, g, p) -> (cores, p, g)
        idx_c = idx.reshape(N_CORES, GPP, 128).transpose(0, 2, 1)
        idx_all[:, :, l * GPP : (l + 1) * GPP] = idx_c.astype(np.int32)
        w_c = w01.reshape(N_CORES, GPP, 128, 18).transpose(0, 2, 1, 3)
        w01_all[:, :, l * GPP * 18 : (l + 1) * GPP * 18] = w_c.reshape(
            N_CORES, 128, -1
        ).astype(bf16)
        v_c = v01.reshape(N_CORES, GPP, 128, 18).transpose(0, 2, 1, 3)
        v01_all[:, :, l * GPP * 18 : (l + 1) * GPP * 18] = v_c.reshape(
            N_CORES, 128, -1
        ).astype(bf16)
    for c in range(N_CORES):
        in_maps[c]["idx"] = idx_all[c]
        in_maps[c]["w01"] = w01_all[c]
        in_maps[c]["v01"] = v01_all[c]
    return in_maps


def kernel(corr0, corr1, corr2, corr3, flow):
    global _prog, LAST_EXEC_NS
    trace = os.environ.get("CORR_TRACE") == "1"
    if trace:
        trace = _install_trace_shim()
    from concourse.bass_utils import run_bass_kernel_spmd

    if _prog is None:
        _prog = _build()
    in_maps = _marshal(corr0, corr1, corr2, corr3, flow)
    res = run_bass_kernel_spmd(
        _prog,
        in_maps,
        core_ids=list(range(N_CORES)),
        trace=trace,
        trace_cores=[0] if trace else None,
    )
    LAST_EXEC_NS = res.exec_time_ns
    if trace and res.instructions_and_trace:
        kernel.last_insts = res.instructions_and_trace
    full = np.empty((N, 324), dtype=np.float32)
    for c in range(N_CORES):
        lo = c * NPX
        for l in range(4):
            o = np.asarray(res.results[c][f"out{l}"]).astype(np.float32)
            o = o.reshape(128, GPP, 81)
            full[lo : lo + NPX, l * 81 : (l + 1) * 81] = (
                o.transpose(1, 0, 2).reshape(NPX, 81)
            )
    return np.ascontiguousarray(
        full.reshape(B, H, W, 324).transpose(0, 3, 1, 2)
    )
